# revision 1
# baseline (speedup 1.0000x reference)
"""Causal multi-head attention block (QKV proj + RoPE + attention + out proj)
for Trainium2, distributed over 8 NeuronCores.

Sharding: B=2 batches x H=16 heads = 32 (b,h) pairs; core c handles batch
c//4 and heads 4*(c%4)..4*(c%4)+3 (data parallel over batch, tensor parallel
over heads: column-parallel Wqkv, row-parallel Wout). Each core emits a
partial [T, C] output; the host sums the 4 partials per batch.

All matmuls run in float32r (full PE rate at free dim >= 256, ~1e-4 matmul
relative error).
"""

import numpy as np

import concourse.bass as bass
import concourse.mybir as mybir
from concourse import bacc
from concourse.tile import TileContext
from concourse.bass_utils import run_bass_kernel_spmd

# Problem shapes (hardcoded per contract).
B, T, C = 2, 2048, 1024
H = 16
HD = C // H            # 64
N_CORES = 8
HEADS_PER_CORE = H // (N_CORES // B)   # 4
HPC = HEADS_PER_CORE
ROT = HD // 2          # 32

F32 = mybir.dt.float32
F32R = mybir.dt.float32r

NT = T // 512          # 4 column tiles of 512 tokens
TT = T // 128          # 16 row tiles of 128 tokens
KC = C // 128          # 8 contraction chunks for the projections


def _r(ap):
    """Matmul operands are already float32r; kept for readability."""
    return ap


def build_kernel():
    nc = bacc.Bacc("TRN2", target_bir_lowering=False, debug=False,
                   num_devices=N_CORES)

    # DRAM I/O (per-core shards, same names on every core)
    xT_d = nc.dram_tensor("xT", [C, T], F32R, kind="ExternalInput")
    wqkvT_d = nc.dram_tensor("wqkvT", [C, 768], F32R, kind="ExternalInput")
    woutT_d = nc.dram_tensor("woutT", [HPC * HD, C], F32R, kind="ExternalInput")
    cos4_d = nc.dram_tensor("cos4", [128, T], F32, kind="ExternalInput")
    sin4_d = nc.dram_tensor("sin4", [128, T], F32, kind="ExternalInput")
    mask_d = nc.dram_tensor("mask", [128, 896], F32R, kind="ExternalInput")
    vones_d = nc.dram_tensor("vones", [128, HPC], F32R, kind="ExternalInput")
    out_d = nc.dram_tensor("out_partial", [T, C], F32, kind="ExternalOutput")

    with TileContext(nc) as tc:
        with (
            tc.tile_pool(name="const", bufs=1) as constp,
            tc.tile_pool(name="rot", bufs=1) as rotp,
            tc.tile_pool(name="vp", bufs=1) as vp,
            tc.tile_pool(name="tmp", bufs=2) as tmpp,
            tc.tile_pool(name="ps_s", bufs=2, space="PSUM") as ps_s,
            tc.tile_pool(name="ps_o", bufs=4, space="PSUM") as ps_o,
        ):
            # HAM warmup: keep PE busy with throwaway fp32 matmuls while the
            # input DMAs stream in, so the real matmuls start at 2.4 GHz.
            wscr = constp.tile([128, 640], F32, tag="wscr")
            nc.vector.memset(wscr[:], 0.0)
            wps = ps_o.tile([128, 128], F32, tag="o", name="wps")
            for _ in range(24):
                nc.tensor.matmul(wps[:], wscr[:, 0:128], wscr[:, 512:640],
                                 start=True, stop=True)

            mask = constp.tile([128, 896], F32R, tag="mask")

            # rotated Q/K pair tiles: rQp[p] holds heads (2p, 2p+1):
            # rows 0-63 = even-halves (32 each), rows 64-127 = odd-halves,
            # so head i's 64 rotated dims are rows [64i .. 64i+64) after
            # combining; contraction per head = rows [64*i_local ... ) --
            # arranged so one K=64 matmul per head works.
            rQp = [rotp.tile([128, T], F32R, tag=f"rQp{i}", name=f"rQp{i}") for i in range(2)]
            rKp = [rotp.tile([128, T], F32R, tag=f"rKp{i}", name=f"rKp{i}") for i in range(2)]
            rot_tiles = {"Q": rQp, "K": rKp}

            # V tiles: one per 128-token block, 4 heads interleaved with a
            # ones column: [v_h0(64) 1 v_h1(64) 1 ...] -> [128, 260]
            vtiles = [vp.tile([128, HPC * 65], F32R, tag=f"v{i}", name=f"v{i}") for i in range(TT)]

            # ---- Phase A: projections ----
            with tc.tile_pool(name="xw", bufs=1) as xwp:
                xT = [xwp.tile([128, T], F32R, tag=f"xT{k}", name=f"xT{k}") for k in range(KC)]
                wq = [xwp.tile([128, 768], F32R, tag=f"wq{k}", name=f"wq{k}") for k in range(KC)]
                cos4 = xwp.tile([128, T], F32, tag="cos4")
                sin4 = xwp.tile([128, T], F32, tag="sin4")
                for k in range(KC):
                    nc.sync.dma_start(out=xT[k][:], in_=xT_d[k * 128:(k + 1) * 128, :])
                    nc.sync.dma_start(out=wq[k][:], in_=wqkvT_d[k * 128:(k + 1) * 128, :])
                nc.sync.dma_start(out=cos4[:], in_=cos4_d[:])
                nc.sync.dma_start(out=sin4[:], in_=sin4_d[:])
                nc.sync.dma_start(out=mask[:], in_=mask_d[:])

                # Q/K projection in E/O layout + rotary.
                # wqkvT columns: [0:128]=QE [128:256]=QO [256:384]=KE [384:512]=KO [512:768]=V
                for qk, pair in rot_tiles.items():
                    ce = 0 if qk == "Q" else 256
                    for nt in range(NT):
                        ts = slice(nt * 512, (nt + 1) * 512)
                        pEO = ps_s.tile([128, 1024], F32, tag="s", name="pEO")
                        pE, pO = pEO[:, 0:512], pEO[:, 512:1024]
                        for k in range(KC):
                            nc.tensor.matmul(pE, wq[k][:, ce:ce + 128],
                                             xT[k][:, ts],
                                             start=(k == 0), stop=(k == KC - 1))
                        for k in range(KC):
                            nc.tensor.matmul(pO, wq[k][:, ce + 128:ce + 256],
                                             xT[k][:, ts],
                                             start=(k == 0), stop=(k == KC - 1))
                        # rotary: E' = E*cos - O*sin ; O' = E*sin + O*cos
                        m1 = tmpp.tile([128, 512], F32, tag="m1")
                        m2 = tmpp.tile([128, 512], F32, tag="m2")
                        m3 = tmpp.tile([128, 512], F32, tag="m3")
                        m4 = tmpp.tile([128, 512], F32, tag="m4")
                        tE = tmpp.tile([128, 512], F32R, tag="tE")
                        tO = tmpp.tile([128, 512], F32R, tag="tO")
                        nc.vector.tensor_mul(m1[:], pE, cos4[:, ts])
                        nc.vector.tensor_mul(m2[:], pO, sin4[:, ts])
                        nc.vector.tensor_sub(tE[:], m1[:], m2[:])
                        nc.vector.tensor_mul(m3[:], pE, sin4[:, ts])
                        nc.vector.tensor_mul(m4[:], pO, cos4[:, ts])
                        nc.vector.tensor_add(tO[:], m3[:], m4[:])
                        # partition-shifting copies into head-contiguous
                        # [E(32)|O(32)] layout: head 2p+i -> pair[p] rows 64i..
                        for pi in range(2):
                            for i in range(2):
                                h = 2 * pi + i
                                nc.sync.dma_start(out=pair[pi][64 * i:64 * i + 32, ts],
                                                  in_=tE[32 * h:32 * h + 32, :])
                                nc.sync.dma_start(out=pair[pi][64 * i + 32:64 * i + 64, ts],
                                                  in_=tO[32 * h:32 * h + 32, :])

                # V projection into interleaved [v | 1] tiles
                for tt in range(TT):
                    pv = ps_o.tile([128, 256], F32, tag="o", name="pv")
                    for k in range(KC):
                        nc.tensor.matmul(pv[:], xT[k][:, tt * 128:(tt + 1) * 128],
                                         wq[k][:, 512:768],
                                         start=(k == 0), stop=(k == KC - 1))
                    vt = vtiles[tt]
                    vtr = vt[:].rearrange("p (h c) -> p h c", c=65)
                    nc.sync.dma_start(out=vtr[:, :, 64:65], in_=vones_d[:].rearrange("p (h o) -> p h o", o=1))
                    nc.vector.tensor_copy(vtr[:, :, 0:64], pv[:].rearrange("p (h c) -> p h c", c=64))

            # ---- late pools reuse the x/w SBUF space ----
            with (
                tc.tile_pool(name="late", bufs=1) as latep,
                tc.tile_pool(name="expp", bufs=8) as expp,
            ):
                woutT = [latep.tile([128, C], F32R, tag=f"woutT{i}", name=f"woutT{i}") for i in range(2)]
                for i in range(2):
                    nc.sync.dma_start(out=woutT[i][:], in_=woutT_d[i * 128:(i + 1) * 128, :])
                # normalized O^T, heads stacked 2 per tile: [128, T] x 2
                otn = [latep.tile([128, T], F32R, tag=f"otn{i}", name=f"otn{i}") for i in range(2)]

                def emit_out_proj(qt):
                    # ---- Phase C: output projection for one q block ----
                    for tt in range(qt * 4, qt * 4 + 4):
                        tsl = slice(tt * 128, (tt + 1) * 128)
                        for n in range(2):
                            ns = slice(n * 512, (n + 1) * 512)
                            pp = ps_s.tile([128, 512], F32, tag="s", name="pp")
                            nc.tensor.matmul(pp[:], otn[0][:, tsl], woutT[0][:, ns],
                                             start=True, stop=False)
                            nc.tensor.matmul(pp[:], otn[1][:, tsl], woutT[1][:, ns],
                                             start=False, stop=True)
                            ot = tmpp.tile([128, 512], F32, tag="ot", bufs=4, name="ot")
                            nc.vector.tensor_copy(ot[:], pp[:])
                            nc.sync.dma_start(out=out_d[tsl, ns], in_=ot[:])

                # ---- Phase B: attention (S^T flash layout, no max subtraction)
                # Two heads per pass so score blocks pipeline in PSUM.
                for qt in range(NT):
                    nkb = 4 * qt + 4
                    for p in range(2):
                        if p == 1 and qt > 0:
                            emit_out_proj(qt - 1)
                        heads = (2 * p, 2 * p + 1)
                        po = [ps_o.tile([65, 512], F32, tag="o", name=f"po{qt}_{p}_{i}")
                              for i in range(2)]

                        def emit_pv(item, heads=heads, po=po, nkb=nkb):
                            kb, off, es = item
                            for i, h in enumerate(heads):
                                nc.tensor.matmul(po[i][:, off:512],
                                                 vtiles[kb][:, 65 * h:65 * h + 65],
                                                 es[:, 512 * i + off:512 * (i + 1)],
                                                 start=(kb == 0), stop=(kb == nkb - 1))

                        esl = []
                        for kb in range(nkb):
                            ks = slice(kb * 128, (kb + 1) * 128)
                            off = max(0, kb * 128 - qt * 512)
                            qs = slice(qt * 512 + off, (qt + 1) * 512)
                            stile = ps_s.tile([128, 1024], F32, tag="s",
                                              name=f"st{qt}_{p}_{kb}")
                            # full-array keeper: garbage matmul immediately
                            # overwritten by the real S matmuls (start=True);
                            # keeps the PE activity monitor at full clock
                            nc.tensor.matmul(stile[:, 0:256], mask[:, 0:128],
                                             mask[:, 128:384], start=True, stop=True)
                            for i in range(2):
                                hs = slice(64 * i, 64 * i + 64)
                                dst = stile[:, 512 * i + off:512 * (i + 1)]
                                nc.tensor.matmul(dst, rKp[p][hs, ks], rQp[p][hs, qs],
                                                 start=True, stop=True,
                                                 tile_position=(64 * i, 0))
                            es = expp.tile([128, 1024], F32R, tag="e",
                                           name=f"es{qt}_{p}_{kb}")
                            if off == 0:
                                nc.scalar.activation(es[:], stile[:],
                                                     mybir.ActivationFunctionType.Exp,
                                                     scale=0.125)
                            else:
                                for i in range(2):
                                    nc.scalar.activation(
                                        es[:, 512 * i + off:512 * (i + 1)],
                                        stile[:, 512 * i + off:512 * (i + 1)],
                                        mybir.ActivationFunctionType.Exp,
                                        scale=0.125)
                            if kb >= 4 * qt:  # diagonal block: causal mask
                                for i in range(2):
                                    sl = es[:, 512 * i + off:512 * (i + 1)]
                                    nc.vector.tensor_mul(sl, sl,
                                                         mask[:, 384:896 - off])
                            esl.append((kb, off, es))
                            # PV trails exp by a couple of blocks so the PE
                            # always has ready work while ACT runs ahead
                            if len(esl) > 2:
                                emit_pv(esl.pop(0))
                        for item in esl:
                            emit_pv(item)
                        # normalize: otn[p][64*i:...] = po[i][0:64] * (1/denom)
                        qs_full = slice(qt * 512, (qt + 1) * 512)
                        for i in range(2):
                            dcopy = tmpp.tile([1, 512], F32, tag="dcopy")
                            nc.vector.tensor_copy(dcopy[:], po[i][64:65, :])
                            rrow = tmpp.tile([1, 512], F32, tag="rrow")
                            nc.vector.reciprocal_approx_fast(rrow[:], dcopy[:])
                            rbc = tmpp.tile([64, 512], F32, tag="rbc")
                            nc.gpsimd.partition_broadcast(rbc[:], rrow[:])
                            dst = otn[p][64 * i:64 * i + 64, qs_full]
                            nc.vector.tensor_mul(dst, po[i][0:64, :], rbc[:])


                emit_out_proj(NT - 1)

    nc.compile()
    return nc


def _prep_core_inputs(x, freqs, Wqkv, Wout, core):
    b = core // (N_CORES // B)
    hg = core % (N_CORES // B)
    heads = [HPC * hg + j for j in range(HPC)]

    xT = np.ascontiguousarray(x[b].T)  # [C, T]

    # Wqkv row permutation: [QE(128) QO(128) KE(128) KO(128) V(256)]
    rows = []
    for which in (0, 1):  # q, k
        for par in (0, 1):  # evens, odds
            for h in heads:
                rows.extend(which * C + h * HD + 2 * i + par for i in range(ROT))
    for h in heads:  # v, natural dim order
        rows.extend(2 * C + h * HD + d for d in range(HD))
    wqkvT = np.ascontiguousarray(Wqkv[rows, :].T)  # [C, 768]

    cols = [h * HD + d for h in heads for d in range(HD)]
    woutT = np.ascontiguousarray(Wout[:, cols].T)  # [256, C]

    cosT = np.cos(freqs).T.astype(np.float32)  # [32, T]
    sinT = np.sin(freqs).T.astype(np.float32)
    cos4 = np.ascontiguousarray(np.tile(cosT, (4, 1)))  # [128, T]
    sin4 = np.ascontiguousarray(np.tile(sinT, (4, 1)))

    p = np.arange(128)[:, None]
    j = np.arange(896)[None, :]
    mask = (p <= j - 384).astype(np.float32)  # [128, 896]

    return {
        "xT": xT.astype(np.float32),
        "wqkvT": wqkvT.astype(np.float32),
        "woutT": woutT.astype(np.float32),
        "cos4": cos4,
        "sin4": sin4,
        "mask": mask,
        "vones": np.ones((128, HPC), dtype=np.float32),
    }


_NC_CACHE = None


def kernel(x, freqs, Wqkv, Wout, _trace=False, _trace_kwargs=None):
    global _NC_CACHE
    x = np.asarray(x, dtype=np.float32)
    freqs = np.asarray(freqs, dtype=np.float32)
    Wqkv = np.asarray(Wqkv, dtype=np.float32)
    Wout = np.asarray(Wout, dtype=np.float32)

    if _NC_CACHE is None:
        _NC_CACHE = build_kernel()
    nc = _NC_CACHE

    in_maps = [_prep_core_inputs(x, freqs, Wqkv, Wout, c) for c in range(N_CORES)]
    res = run_bass_kernel_spmd(nc, in_maps, core_ids=list(range(N_CORES)),
                               trace=_trace, **(_trace_kwargs or {}))

    out = np.empty((B, T, C), dtype=np.float32)
    gpb = N_CORES // B
    for b in range(B):
        acc = res.results[b * gpb]["out_partial"].astype(np.float32)
        for c in range(b * gpb + 1, (b + 1) * gpb):
            acc = acc + res.results[c]["out_partial"]
        out[b] = acc
    kernel._last_results = res
    return out



# revision 6
# speedup vs baseline: 1.3773x; 1.3773x over previous
"""Causal multi-head attention block (QKV proj + RoPE + attention + out proj)
for Trainium2, distributed over 8 NeuronCores.

Sharding: B=2 batches x H=16 heads = 32 (b,h) pairs; core c handles batch
c//4 and heads 4*(c%4)..4*(c%4)+3 (data parallel over batch, tensor parallel
over heads: column-parallel Wqkv, row-parallel Wout). Each core emits a
partial [T, C] output; the host sums the 4 partials per batch.

v2: bf16 operands (fp32 psum accumulation), and a software-pipelined
emission schedule: projection of token-block nt+1 and the output projection
of block qt-1 are interleaved as filler work between the attention blocks of
qt, so the PE activity monitor keeps the clock at 2.4 GHz without throwaway
keeper matmuls.
"""

import numpy as np
import ml_dtypes

import concourse.bass as bass
import concourse.mybir as mybir
from concourse import bacc
from concourse.tile import TileContext
from concourse.bass_utils import run_bass_kernel_spmd

# Problem shapes (hardcoded per contract).
B, T, C = 2, 2048, 1024
H = 16
HD = C // H            # 64
N_CORES = 8
HEADS_PER_CORE = H // (N_CORES // B)   # 4
HPC = HEADS_PER_CORE
ROT = HD // 2          # 32

F32 = mybir.dt.float32
BF16 = mybir.dt.bfloat16

NT = T // 512          # 4 column tiles of 512 tokens
TT = T // 128          # 16 row tiles of 128 tokens
KC = C // 128          # 8 contraction chunks for the projections

BF = ml_dtypes.bfloat16


def build_kernel():
    nc = bacc.Bacc("TRN2", target_bir_lowering=False, debug=False,
                   num_devices=N_CORES)

    # DRAM I/O (per-core shards, same names on every core)
    xT_d = nc.dram_tensor("xT", [C, T], BF16, kind="ExternalInput")
    wqkvT_d = nc.dram_tensor("wqkvT", [C, 768], BF16, kind="ExternalInput")
    woutT_d = nc.dram_tensor("woutT", [HPC * HD, C], BF16, kind="ExternalInput")
    cos4_d = nc.dram_tensor("cos4", [128, T], F32, kind="ExternalInput")
    sin4_d = nc.dram_tensor("sin4", [128, T], F32, kind="ExternalInput")
    mask2_d = nc.dram_tensor("mask2", [128, 1024], BF16, kind="ExternalInput")
    vones_d = nc.dram_tensor("vones", [128, HPC], BF16, kind="ExternalInput")
    out_d = nc.dram_tensor("out_partial", [T, C], F32, kind="ExternalOutput")

    with TileContext(nc) as tc:
        with (
            tc.tile_pool(name="const", bufs=1) as constp,
            tc.tile_pool(name="rot", bufs=1) as rotp,
            tc.tile_pool(name="vp", bufs=1) as vp,
            tc.tile_pool(name="xw", bufs=1) as xwp,
            tc.tile_pool(name="late", bufs=1) as latep,
            tc.tile_pool(name="tmp", bufs=2) as tmpp,
            tc.tile_pool(name="expp", bufs=8) as expp,
            tc.tile_pool(name="ps_s", bufs=2, space="PSUM") as ps_s,
            tc.tile_pool(name="ps_o", bufs=2, space="PSUM") as ps_o,
            tc.tile_pool(name="ps_w", bufs=2, space="PSUM") as ps_w,
        ):
            mask2 = constp.tile([128, 1024], BF16, tag="mask2")
            cos4 = constp.tile([128, T], F32, tag="cos4")
            sin4 = constp.tile([128, T], F32, tag="sin4")
            wscr = constp.tile([128, 512], BF16, tag="wscr")

            xT = [xwp.tile([128, T], BF16, tag=f"xT{k}", name=f"xT{k}")
                  for k in range(KC)]
            wq = [xwp.tile([128, 768], BF16, tag=f"wq{k}", name=f"wq{k}")
                  for k in range(KC)]

            # rotated Q/K pair tiles: index p holds heads (2p, 2p+1); head
            # rows: [E(32)|O(32)] per head -> head i at rows 64i..64i+64.
            rQp = [rotp.tile([128, T], BF16, tag=f"rQp{i}", name=f"rQp{i}")
                   for i in range(2)]
            rKp = [rotp.tile([128, T], BF16, tag=f"rKp{i}", name=f"rKp{i}")
                   for i in range(2)]
            rot_tiles = {"Q": rQp, "K": rKp}

            # V tiles: per 128-token block, 4 heads interleaved with a ones
            # column: [v_h0(64) 1 v_h1(64) 1 ...] -> [128, 260]
            vtiles = [vp.tile([128, HPC * 65], BF16, tag=f"v{i}", name=f"v{i}")
                      for i in range(TT)]

            woutT = [latep.tile([128, C], BF16, tag=f"woutT{i}", name=f"woutT{i}")
                     for i in range(2)]
            # normalized O^T, heads stacked 2 per tile: [128, T] x 2
            otn = [latep.tile([128, T], BF16, tag=f"otn{i}", name=f"otn{i}")
                   for i in range(2)]

            # ---- input DMAs (emission order = rough priority) ----
            nc.vector.memset(wscr[:], 0.0)
            nc.sync.dma_start(out=mask2[:], in_=mask2_d[:])
            nc.sync.dma_start(out=cos4[:], in_=cos4_d[:])
            nc.sync.dma_start(out=sin4[:], in_=sin4_d[:])
            for k in range(KC):
                nc.sync.dma_start(out=wq[k][:], in_=wqkvT_d[k * 128:(k + 1) * 128, :])
                nc.sync.dma_start(out=xT[k][:, 0:512],
                                  in_=xT_d[k * 128:(k + 1) * 128, 0:512])
            for i in range(2):
                nc.sync.dma_start(out=woutT[i][:], in_=woutT_d[i * 128:(i + 1) * 128, :])
            for nt in range(1, NT):
                ts = slice(nt * 512, (nt + 1) * 512)
                for k in range(KC):
                    nc.sync.dma_start(out=xT[k][:, ts],
                                      in_=xT_d[k * 128:(k + 1) * 128, ts])

            # ---------------- emission-unit builders ----------------

            def emit_keeper():
                # PE activity filler during the initial DMA wait; writes a
                # throwaway psum tile nothing reads.
                kp = ps_s.tile([128, 1024], F32, tag="s", name="keep")
                nc.tensor.matmul(kp[:, 0:256], wscr[:, 0:128], wscr[:, 0:256],
                                 start=True, stop=True)

            def proj_units(nt):
                """Filler units projecting token block nt: Q/K E/O chains +
                rotary + head-shuffle, then V chains. Each unit is a closure."""
                ts = slice(nt * 512, (nt + 1) * 512)
                units = []
                for qk in ("Q", "K"):
                    ce = 0 if qk == "Q" else 256
                    state = {}

                    def chain(par, ce=ce, ts=ts, state=state):
                        p = ps_w.tile([128, 512], F32, tag="w",
                                      name=f"pc{ce}{par}")
                        cs = ce + 128 * par
                        for k in range(KC):
                            nc.tensor.matmul(p[:], wq[k][:, cs:cs + 128],
                                             xT[k][:, ts],
                                             start=(k == 0), stop=(k == KC - 1))
                        state[par] = p

                    def rotary(qk=qk, ts=ts, state=state):
                        pE, pO = state[0][:], state[1][:]
                        pair = rot_tiles[qk]
                        m1 = tmpp.tile([128, 512], F32, tag="m1")
                        m2 = tmpp.tile([128, 512], F32, tag="m2")
                        m3 = tmpp.tile([128, 512], F32, tag="m3")
                        m4 = tmpp.tile([128, 512], F32, tag="m4")
                        tE = tmpp.tile([128, 512], BF16, tag="tE")
                        tO = tmpp.tile([128, 512], BF16, tag="tO")
                        nc.vector.tensor_mul(m1[:], pE, cos4[:, ts])
                        nc.vector.tensor_mul(m2[:], pO, sin4[:, ts])
                        nc.vector.tensor_sub(tE[:], m1[:], m2[:])
                        nc.vector.tensor_mul(m3[:], pE, sin4[:, ts])
                        nc.vector.tensor_mul(m4[:], pO, cos4[:, ts])
                        nc.vector.tensor_add(tO[:], m3[:], m4[:])
                        # partition-shifting copies into head-contiguous
                        # [E(32)|O(32)] per-head layout
                        for pi in range(2):
                            for i in range(2):
                                h = 2 * pi + i
                                nc.sync.dma_start(
                                    out=pair[pi][64 * i:64 * i + 32, ts],
                                    in_=tE[32 * h:32 * h + 32, :])
                                nc.sync.dma_start(
                                    out=pair[pi][64 * i + 32:64 * i + 64, ts],
                                    in_=tO[32 * h:32 * h + 32, :])

                    units.append(lambda chain=chain: chain(0))
                    units.append(lambda chain=chain: chain(1))
                    units.append(rotary)

                for tt in range(nt * 4, nt * 4 + 4):
                    def vchain(tt=tt):
                        pv = ps_w.tile([128, 256], F32, tag="w", name=f"pv{tt}")
                        for k in range(KC):
                            nc.tensor.matmul(pv[:],
                                             xT[k][:, tt * 128:(tt + 1) * 128],
                                             wq[k][:, 512:768],
                                             start=(k == 0), stop=(k == KC - 1))
                        vt = vtiles[tt]
                        vtr = vt[:].rearrange("p (h c) -> p h c", c=65)
                        nc.sync.dma_start(
                            out=vtr[:, :, 64:65],
                            in_=vones_d[:].rearrange("p (h o) -> p h o", o=1))
                        nc.vector.tensor_copy(
                            vtr[:, :, 0:64],
                            pv[:].rearrange("p (h c) -> p h c", c=64))
                    units.append(vchain)
                return units

            def outproj_units(qt):
                """Output projection for q block qt: 8 units of (tt, n)."""
                units = []
                for tt in range(qt * 4, qt * 4 + 4):
                    for n in range(2):
                        def u(tt=tt, n=n):
                            tsl = slice(tt * 128, (tt + 1) * 128)
                            ns = slice(n * 512, (n + 1) * 512)
                            pp = ps_w.tile([128, 512], F32, tag="w", name="pp")
                            nc.tensor.matmul(pp[:], otn[0][:, tsl],
                                             woutT[0][:, ns],
                                             start=True, stop=False)
                            nc.tensor.matmul(pp[:], otn[1][:, tsl],
                                             woutT[1][:, ns],
                                             start=False, stop=True)
                            ot = tmpp.tile([128, 512], F32, tag="ot", bufs=4,
                                           name="ot")
                            nc.vector.tensor_copy(ot[:], pp[:])
                            nc.sync.dma_start(out=out_d[tsl, ns], in_=ot[:])
                        units.append(u)
                return units

            # ---------------- attention stage for one q block ----------------

            def attention_stage(qt, units):
                nkb = 4 * qt + 4
                blocks_total = 2 * nkb
                emitted = [0]
                bcount = [0]

                def pump(force=False):
                    if force:
                        tgt = len(units)
                    else:
                        tgt = (len(units) * bcount[0]) // blocks_total
                    while emitted[0] < tgt:
                        units[emitted[0]]()
                        emitted[0] += 1

                for p in range(2):
                    heads = (2 * p, 2 * p + 1)
                    po = [ps_o.tile([65, 512], F32, tag="o",
                                    name=f"po{qt}_{p}_{i}") for i in range(2)]

                    def emit_pv(item, heads=heads, po=po, nkb=nkb):
                        kb, off, es = item
                        for i, h in enumerate(heads):
                            nc.tensor.matmul(po[i][:, off:512],
                                             vtiles[kb][:, 65 * h:65 * h + 65],
                                             es[:, 512 * i + off:512 * (i + 1)],
                                             start=(kb == 0), stop=(kb == nkb - 1))

                    esl = []
                    for kb in range(nkb):
                        ks = slice(kb * 128, (kb + 1) * 128)
                        off = max(0, kb * 128 - qt * 512)
                        qs = slice(qt * 512 + off, (qt + 1) * 512)
                        stile = ps_s.tile([128, 1024], F32, tag="s",
                                          name=f"st{qt}_{p}_{kb}")
                        for i in range(2):
                            hs = slice(64 * i, 64 * i + 64)
                            dst = stile[:, 512 * i + off:512 * (i + 1)]
                            nc.tensor.matmul(dst, rKp[p][hs, ks], rQp[p][hs, qs],
                                             start=True, stop=True,
                                             tile_position=(64 * i, 0))
                        es = expp.tile([128, 1024], BF16, tag="e",
                                       name=f"es{qt}_{p}_{kb}")
                        if off == 0:
                            nc.scalar.activation(es[:], stile[:],
                                                 mybir.ActivationFunctionType.Exp,
                                                 scale=0.125)
                        else:
                            # one strided-AP instruction over both heads'
                            # valid columns
                            esr = es[:].rearrange("p (i w) -> p i w", i=2)
                            srr = stile[:].rearrange("p (i w) -> p i w", i=2)
                            nc.scalar.activation(esr[:, :, off:512],
                                                 srr[:, :, off:512],
                                                 mybir.ActivationFunctionType.Exp,
                                                 scale=0.125)
                        if kb >= 4 * qt:  # diagonal block: causal mask
                            # mask[p, c] = (p <= c) lines up with es column
                            # j = off + c, so the mask slice starts at 0
                            esr = es[:].rearrange("p (i w) -> p i w", i=2)
                            mkr = mask2[:].rearrange("p (i w) -> p i w", i=2)
                            nc.vector.tensor_mul(esr[:, :, off:512],
                                                 esr[:, :, off:512],
                                                 mkr[:, :, 0:512 - off])
                        esl.append((kb, off, es))
                        bcount[0] += 1
                        pump()
                        # PV trails exp by a couple of blocks so the PE
                        # always has ready work while ACT runs ahead
                        if len(esl) > 2:
                            emit_pv(esl.pop(0))
                    for item in esl:
                        emit_pv(item)

                    # normalize: otn[p][64*i:...] = po[i][0:64] * (1/denom)
                    qs_full = slice(qt * 512, (qt + 1) * 512)
                    for i in range(2):
                        dcopy = tmpp.tile([1, 512], F32, tag="dcopy")
                        nc.vector.tensor_copy(dcopy[:], po[i][64:65, :])
                        rrow = tmpp.tile([1, 512], F32, tag="rrow")
                        nc.vector.reciprocal_approx_fast(rrow[:], dcopy[:])
                        rbc = tmpp.tile([64, 512], F32, tag="rbc")
                        nc.gpsimd.partition_broadcast(rbc[:], rrow[:])
                        dst = otn[p][64 * i:64 * i + 64, qs_full]
                        nc.vector.tensor_mul(dst, po[i][0:64, :], rbc[:])

                pump(force=True)

            # ---------------- top-level schedule ----------------

            # stage -1: initial projection of nt=0 with keepers while the
            # first DMAs stream in
            units0 = proj_units(0)
            for u in units0:
                emit_keeper()
                emit_keeper()
                u()

            for qt in range(NT):
                units = []
                if qt < NT - 1:
                    units += proj_units(qt + 1)
                if qt > 0:
                    units += outproj_units(qt - 1)
                attention_stage(qt, units)

            for u in outproj_units(NT - 1):
                u()

    nc.compile()
    return nc


def _prep_core_inputs(x, freqs, Wqkv, Wout, core):
    b = core // (N_CORES // B)
    hg = core % (N_CORES // B)
    heads = [HPC * hg + j for j in range(HPC)]

    xT = np.ascontiguousarray(x[b].T).astype(BF)  # [C, T]

    # Wqkv row permutation: [QE(128) QO(128) KE(128) KO(128) V(256)]
    rows = []
    for which in (0, 1):  # q, k
        for par in (0, 1):  # evens, odds
            for h in heads:
                rows.extend(which * C + h * HD + 2 * i + par for i in range(ROT))
    for h in heads:  # v, natural dim order
        rows.extend(2 * C + h * HD + d for d in range(HD))
    wqkvT = np.ascontiguousarray(Wqkv[rows, :].T).astype(BF)  # [C, 768]

    cols = [h * HD + d for h in heads for d in range(HD)]
    woutT = np.ascontiguousarray(Wout[:, cols].T).astype(BF)  # [256, C]

    cosT = np.cos(freqs).T.astype(np.float32)  # [32, T]
    sinT = np.sin(freqs).T.astype(np.float32)
    cos4 = np.ascontiguousarray(np.tile(cosT, (4, 1)))  # [128, T]
    sin4 = np.ascontiguousarray(np.tile(sinT, (4, 1)))

    p = np.arange(128)[:, None]
    j = np.arange(512)[None, :]
    mask = (p <= j).astype(BF)  # [128, 512]
    mask2 = np.ascontiguousarray(np.concatenate([mask, mask], axis=1))

    return {
        "xT": xT,
        "wqkvT": wqkvT,
        "woutT": woutT,
        "cos4": cos4,
        "sin4": sin4,
        "mask2": mask2,
        "vones": np.ones((128, HPC), dtype=BF),
    }


_NC_CACHE = None


def kernel(x, freqs, Wqkv, Wout, _trace=False, _trace_kwargs=None):
    global _NC_CACHE
    x = np.asarray(x, dtype=np.float32)
    freqs = np.asarray(freqs, dtype=np.float32)
    Wqkv = np.asarray(Wqkv, dtype=np.float32)
    Wout = np.asarray(Wout, dtype=np.float32)

    if _NC_CACHE is None:
        _NC_CACHE = build_kernel()
    nc = _NC_CACHE

    in_maps = [_prep_core_inputs(x, freqs, Wqkv, Wout, c) for c in range(N_CORES)]
    res = run_bass_kernel_spmd(nc, in_maps, core_ids=list(range(N_CORES)),
                               trace=_trace, **(_trace_kwargs or {}))

    out = np.empty((B, T, C), dtype=np.float32)
    gpb = N_CORES // B
    for b in range(B):
        acc = res.results[b * gpb]["out_partial"].astype(np.float32)
        for c in range(b * gpb + 1, (b + 1) * gpb):
            acc = acc + res.results[c]["out_partial"]
        out[b] = acc
    kernel._last_results = res
    return out
